# revision 16
# baseline (speedup 1.0000x reference)
"""Entropic OT (Sinkhorn) kernel for Trainium2, 8 NeuronCores.

Math summary
------------
reference() computes, in float32:
    C      = ||x_i - y_j||^2                       [N, N]
    log_K  = -C / 0.1 = -10*C
    100 Sinkhorn iterations of
        log_u = log_a - LSE_i(log_K[i,j] + v[i]);  u = exp(log_u)
        log_v = log_b - LSE_i(log_K[j,i] + u[i]);  v = exp(log_v)
    plan   = exp(log_K + v[:,None] + u[None,:])

For this input (N=8192, D=64, unit gaussians) min_ij C ~ 24.5, so
LSE_i(log_K[i,j] + 0) <= -10*min_i C[i,j] + ln(N) and
log_u[j] >= log_a + 10*min(C) - ln(N) >= 226, far above the f32 exp
overflow point (88.73).  Hence u == +inf for every j at iteration 0,
which forces v == 0 (jax's logsumexp returns +inf for a column
containing +inf), and (u=+inf, v=0) is a bitwise fixed point of the
iteration (verified against the reference).  Therefore
plan == exp(finite + 0 + inf) == +inf everywhere and the only
nontrivial output is C itself.

The device kernel computes C, row-sharded across the 8 cores (core d
owns rows [d*1024, (d+1)*1024)).  The host then verifies the
saturation bound rigorously from the returned C and emits the plan; a
faithful numpy Sinkhorn fallback covers the case the bound fails
(impossible for this input, but kept for safety).

Device kernel
-------------
C = x2[:,None] + y2[None,:] - 2*X@Y.T as one matmul by augmenting the
contraction (a = -2*X):
    out[m,n] = sum_d a[m,d]*y[n,d] + 1*y2[n] + x2[m]*1 = C[m,n]

TRN2 fp32 matmul is a 2-pass HI/LO emulation (~1060ns per pass at
N=512), so instead each f32 operand is split into two bf16 parts
(a = ah + al exactly to 16 mantissa bits) and the three significant
cross products are accumulated in f32 PSUM:
    a.y ~= ah.yh + ah.yl + al.yh        (drops al*yl ~ 2^-16 |a||y|)
and the PE streams one column per cycle regardless of contraction
depth, so the two cross terms are STACKED into one K=128 matmul:
    pass 1 (K=70):  lhsT=[ah^T; ones; x2 splits] rhs=[yh^T; y2 splits; ones]
    pass 2 (K=128): lhsT=[ah^T; al^T]            rhs=[yl^T; yh^T]
Two bf16 passes are ~4x cheaper than one fp32 matmul (which is a
2-pass HI/LO emulation at 4 cycles/column) and accurate to ~1e-5
relative here (validated: max elementwise rel err 7e-6 on this input).
The y2/x2 augment lanes use exact 3-way bf16 splits multiplied by
exact ones, so the augment contributes error-free.

All operands ship pre-split/pre-transposed from the host (O(N*D)
layout prep) via SWDGE DMA pieces issued by the otherwise-idle GpSimd
engine, x-parts first so the PE starts after one piece per block.  The
kernel is raw bass: the TRN2 LDWEIGHTS encoding has only one
sync-wait slot, so Tile's auto-semaphores (which attach {PE, DVE} wait
pairs to matmuls on PSUM slot reuse) fail walrus codegen; standalone
wait_ge instructions avoid the limit.

Pipeline per core: 8 row strips of 128; per strip 4 "quads" of
[128, 2048] PSUM (4 banks); per quad 4 x 2 bf16 matmuls; quads are
evacuated PSUM->SBUF alternately by the vector and scalar engines into
4 rotating [128, 8192] strips, each written to HBM as two contiguous
2 MB half-strip DMAs.  Per-core span ~110-125 us, close to the HBM
floor (36.7 MB of traffic at ~400 GB/s plus ~8 us NEFF preamble).
"""

import numpy as np
import ml_dtypes

import concourse.bass as bass
import concourse.mybir as mybir
from concourse.bass_utils import run_bass_kernel_spmd

N = 8192
D = 64
NCORES = 8
NL = N // NCORES          # 1024 rows of C per core
KB = D + 6                # 70: 64 data rows + 3 y2-split + 3 x2-split lanes (pass 1)
EPSILON = 0.1
F32 = mybir.dt.float32
BF16 = mybir.dt.bfloat16
NPBF16 = ml_dtypes.bfloat16

RSTRIPS = NL // 128       # 8 row strips of 128
QW = 2048                 # quad width: 4 PSUM banks, evacuated in one op
NQ = N // QW              # 4 quads per strip
MMW = 512                 # matmul moving width (one f32 PSUM bank)

# blockA [128, NL+N]: [ah_aug | yh_aug] rows 0-69 (70-127 zero padding
#                     so the DMA spreads over all 128 partitions)
# blockB [128, NL+N]: [ah;al  | yl;yh ]  (pass-2 stacked cross operands)
# x-part first: input piece 0 = x-part + first y-quarter in one DMA.
BLK_COLS = NL + N

_cached_nc = None


def _build_nc():
    nc = bass.Bass()

    blk_a = nc.declare_dram_parameter("blk_a", [128, BLK_COLS], BF16,
                                      isOutput=False)
    blk_b = nc.declare_dram_parameter("blk_b", [128, BLK_COLS], BF16,
                                      isOutput=False)
    c_out = nc.declare_dram_parameter("c_out", [NL, N], F32, isOutput=True)

    from contextlib import ExitStack
    ctx = ExitStack()
    with ctx:
        sb_a = ctx.enter_context(nc.sbuf_tensor("sb_a", [128, BLK_COLS], BF16))
        sb_b = ctx.enter_context(nc.sbuf_tensor("sb_b", [128, BLK_COLS], BF16))
        strips_h = [
            ctx.enter_context(nc.sbuf_tensor(f"strip{i}", [128, N], F32))
            for i in range(4)
        ]
        ps0 = ctx.enter_context(nc.psum_tensor("ps0", [128, QW], F32))
        ps1 = ctx.enter_context(nc.psum_tensor("ps1", [128, QW], F32))
        # NOTE on DMA semaphores: increments from concurrent DMAs on one
        # ring interleave (each contributes 16 in engine-completion order),
        # so a wait value is only sound if it equals the TOTAL of all
        # DMA increments that can possibly have been issued at wait time.
        # Hence: one semaphore per input piece, and one output semaphore
        # per strip buffer (consecutive users of a buffer are 4 strips
        # apart, so the wait total covers exactly the prior users).
        s_aq = [ctx.enter_context(nc.semaphore(f"s_a{j}")) for j in range(NQ)]
        s_bq = [ctx.enter_context(nc.semaphore(f"s_b{j}")) for j in range(NQ)]
        s_mm = ctx.enter_context(nc.semaphore("s_mm"))
        s_ev_d = ctx.enter_context(nc.semaphore("s_ev_d"))  # DVE evacs
        s_ev_a = ctx.enter_context(nc.semaphore("s_ev_a"))  # ACT evacs
        s_out = [ctx.enter_context(nc.semaphore(f"s_out{b}")) for b in range(4)]
        block = ctx.enter_context(nc.Block())

        strips = strips_h
        psb = [ps0, ps1]
        HW = N // 2               # half-strip output DMA width

        def piece(j):
            """Input piece j: x-part + y-quarter 0 for j=0, else quarter j."""
            lo = 0 if j == 0 else NL + j * QW
            hi = NL + (j + 1) * QW
            return lo, hi

        @block.sync
        def _(sync):
            for r in range(RSTRIPS):
                for h in range(2):
                    sync.wait_ge(s_ev_d, 2 * r + h + 1)
                    sync.wait_ge(s_ev_a, 2 * r + h + 1)
                    sync.dma_start(
                        c_out[r * 128:(r + 1) * 128, h * HW:(h + 1) * HW],
                        strips[r % 4][:, h * HW:(h + 1) * HW],
                    ).then_inc(s_out[r % 4], 16)
            for b in range(4):
                nstrips_b = len([r for r in range(RSTRIPS) if r % 4 == b])
                sync.wait_ge(s_out[b], 32 * nstrips_b)

        @block.gpsimd
        def _(gpsimd):
            # input pieces via SWDGE on the otherwise-idle GpSimd engine --
            # it starts issuing while the other engines are still in their
            # NEFF preamble.  Piece 0 carries the x-parts so the PE can
            # start after a single piece per block.
            for j in range(NQ):
                lo, hi = piece(j)
                gpsimd.dma_start(sb_a[:, lo:hi],
                                 blk_a[:, lo:hi]).then_inc(s_aq[j], 16)
                gpsimd.dma_start(sb_b[:, lo:hi],
                                 blk_b[:, lo:hi]).then_inc(s_bq[j], 16)

        @block.tensor
        def _(tensor):
            for r in range(RSTRIPS):
                ah_aug = sb_a[0:KB, r * 128:(r + 1) * 128]
                axl = sb_b[:, r * 128:(r + 1) * 128]
                for q in range(NQ):
                    k = r * NQ + q
                    if r == 0:
                        tensor.wait_ge(s_aq[q], 16)
                        tensor.wait_ge(s_bq[q], 16)
                    if k >= 2:
                        # ps[k%2] must be evacuated (quad k-2, same engine)
                        tensor.wait_ge([s_ev_d, s_ev_a][k % 2], k // 2)
                    ps = psb[k % 2]
                    for s in range(QW // MMW):
                        c0 = NL + q * QW + s * MMW
                        c1 = c0 + MMW
                        out = ps[:, s * MMW:(s + 1) * MMW]
                        tensor.matmul(out, ah_aug, sb_a[0:KB, c0:c1],
                                      start=True, stop=False)
                        mm = tensor.matmul(out, axl, sb_b[:, c0:c1],
                                           start=False, stop=True)
                    mm.then_inc(s_mm, 1)  # matmuls retire in order

        @block.vector
        def _(vector):
            for r in range(RSTRIPS):
                for q in (0, 2):
                    k = r * NQ + q
                    if r >= 4 and q == 0:
                        # strip buffer r%4: previous users are strips
                        # r-4, r-8, ... = r//4 strips, 32 incs each
                        vector.wait_ge(s_out[r % 4], 32 * (r // 4))
                    vector.wait_ge(s_mm, k + 1)
                    vector.tensor_copy(
                        strips[r % 4][:, q * QW:(q + 1) * QW], ps0[:, :]
                    ).then_inc(s_ev_d, 1)

        @block.scalar
        def _(scalar):
            for r in range(RSTRIPS):
                for q in (1, 3):
                    k = r * NQ + q
                    if r >= 4 and q == 1:
                        scalar.wait_ge(s_out[r % 4], 32 * (r // 4))
                    scalar.wait_ge(s_mm, k + 1)
                    scalar.copy(
                        strips[r % 4][:, q * QW:(q + 1) * QW], ps1[:, :]
                    ).then_inc(s_ev_a, 1)

    return nc


def _get_nc():
    global _cached_nc
    if _cached_nc is None:
        _cached_nc = _build_nc()
    return _cached_nc


def _split2(v):
    """Exact 2-way bf16 split: v ~= h + l to 16 mantissa bits."""
    h = v.astype(NPBF16).astype(np.float32)
    l = (v - h).astype(NPBF16).astype(np.float32)
    return h, l


def _split3(v):
    """3-way bf16 split: v == h + m + l to 24 mantissa bits (f32-exact)."""
    h = v.astype(NPBF16).astype(np.float32)
    r = (v - h).astype(np.float32)
    m = r.astype(NPBF16).astype(np.float32)
    l = (r - m).astype(NPBF16).astype(np.float32)
    return h, m, l


def _make_blocks(x_local, y, x2_local, y2):
    """Build blockA and blockB [128, NL+N] bf16 operands (x-part first)."""
    a = (-2.0 * x_local).astype(np.float32)
    ah, al = _split2(a)
    yhi, ylo = _split2(y)
    y2h, y2m, y2l = _split3(y2)
    x2h, x2m, x2l = _split3(x2_local)

    blk_a = np.zeros((128, BLK_COLS), dtype=np.float32)
    # ah_aug (x-part): data, ones (match y2 lanes), x2 splits
    blk_a[:D, :NL] = ah.T
    blk_a[D + 0:D + 3, :NL] = 1.0
    blk_a[D + 3, :NL] = x2h
    blk_a[D + 4, :NL] = x2m
    blk_a[D + 5, :NL] = x2l
    # yh_aug (y-part): data, y2 splits, ones
    blk_a[:D, NL:] = yhi.T
    blk_a[D + 0, NL:] = y2h
    blk_a[D + 1, NL:] = y2m
    blk_a[D + 2, NL:] = y2l
    blk_a[D + 3:D + 6, NL:] = 1.0

    blk_b = np.empty((128, BLK_COLS), dtype=np.float32)
    # stacked cross operands: sum_k lhsT[k]*rhs[k] = ah.yl + al.yh
    blk_b[:D, :NL] = ah.T
    blk_b[D:, :NL] = al.T
    blk_b[:D, NL:] = ylo.T
    blk_b[D:, NL:] = yhi.T
    return blk_a.astype(NPBF16), blk_b.astype(NPBF16)


def _lse_cols(a):
    """jax.nn.logsumexp(a, axis=0) in f32 numpy, with jax's non-finite
    amax guard."""
    amax = np.max(a, axis=0)
    amax_f = np.where(np.isfinite(amax), amax, np.float32(0)).astype(np.float32)
    with np.errstate(over="ignore", invalid="ignore", divide="ignore"):
        s = np.sum(np.exp((a - amax_f[None, :]).astype(np.float32)),
                   axis=0).astype(np.float32)
        return (np.log(s) + amax_f).astype(np.float32)


def _sinkhorn_host(C, max_iter=100):
    """Faithful numpy replication of the reference Sinkhorn loop, used only
    if the saturation fast path does not apply."""
    n = C.shape[0]
    log_K = (-C / np.float32(EPSILON)).astype(np.float32)
    log_a = np.float32(np.log(np.float32(1.0 / n) + np.float32(1e-8)))
    log_b = log_a
    u = np.zeros(n, np.float32)
    v = np.zeros(n, np.float32)
    with np.errstate(over="ignore", invalid="ignore", divide="ignore"):
        for _ in range(max_iter):
            u_new = np.exp((log_a - _lse_cols(log_K + v[:, None])).astype(np.float32))
            v_new = np.exp((log_b - _lse_cols(log_K.T + u_new[:, None])).astype(np.float32))
            if (np.array_equal(u_new, u, equal_nan=True)
                    and np.array_equal(v_new, v, equal_nan=True)):
                u, v = u_new, v_new
                break  # bitwise fixed point: remaining iterations are no-ops
            u, v = u_new, v_new
        plan = np.exp((log_K + v[:, None] + u[None, :]).astype(np.float32))
    return plan


def kernel(**inputs):
    x = np.ascontiguousarray(np.asarray(inputs["source"]), dtype=np.float32)
    y = np.ascontiguousarray(np.asarray(inputs["target"]), dtype=np.float32)
    assert x.shape == (N, D) and y.shape == (N, D)

    x2 = np.sum(x * x, axis=-1, dtype=np.float32)
    y2 = np.sum(y * y, axis=-1, dtype=np.float32)

    nc = _get_nc()
    in_maps = []
    for d in range(NCORES):
        blk_a, blk_b = _make_blocks(x[d * NL:(d + 1) * NL], y,
                                    x2[d * NL:(d + 1) * NL], y2)
        in_maps.append({"blk_a": blk_a, "blk_b": blk_b})
    res = run_bass_kernel_spmd(nc, in_maps, core_ids=list(range(NCORES)))
    C = np.concatenate([r["c_out"] for r in res.results], axis=0)

    # Saturation fast path, rigorous bound: for every column j,
    #   log_u[j] = log_a - LSE_i(-10 C[i,j]) >= log_a + 10*min(C) - ln(N).
    # If that exceeds 100 (f32 exp overflows to +inf at 88.73), then
    # u == +inf everywhere, so v == 0 (LSE of a +inf column is +inf
    # under jax's guard) and (u, v) is a bitwise fixed point
    # => plan = exp(finite + 0 + inf) = +inf everywhere.
    log_a = float(np.log(np.float32(1.0 / N) + np.float32(1e-8)))
    bound = log_a + (1.0 / EPSILON) * float(C.min()) - float(np.log(N))
    if bound > 100.0:
        plan = np.full((N, N), np.inf, dtype=np.float32)
    else:
        plan = _sinkhorn_host(C)
    return plan, C


# revision 17
# speedup vs baseline: 1.0088x; 1.0088x over previous
"""Entropic OT (Sinkhorn) kernel for Trainium2, 8 NeuronCores.

Math summary
------------
reference() computes, in float32:
    C      = ||x_i - y_j||^2                       [N, N]
    log_K  = -C / 0.1 = -10*C
    100 Sinkhorn iterations of
        log_u = log_a - LSE_i(log_K[i,j] + v[i]);  u = exp(log_u)
        log_v = log_b - LSE_i(log_K[j,i] + u[i]);  v = exp(log_v)
    plan   = exp(log_K + v[:,None] + u[None,:])

For this input (N=8192, D=64, unit gaussians) min_ij C ~ 24.5, so
LSE_i(log_K[i,j] + 0) <= -10*min_i C[i,j] + ln(N) and
log_u[j] >= log_a + 10*min(C) - ln(N) >= 226, far above the f32 exp
overflow point (88.73).  Hence u == +inf for every j at iteration 0,
which forces v == 0 (jax's logsumexp returns +inf for a column
containing +inf), and (u=+inf, v=0) is a bitwise fixed point of the
iteration (verified against the reference).  Therefore
plan == exp(finite + 0 + inf) == +inf everywhere and the only
nontrivial output is C itself.

The device kernel computes C, row-sharded across the 8 cores (core d
owns rows [d*1024, (d+1)*1024)).  The host then verifies the
saturation bound rigorously from the returned C and emits the plan; a
faithful numpy Sinkhorn fallback covers the case the bound fails
(impossible for this input, but kept for safety).

Device kernel
-------------
C = x2[:,None] + y2[None,:] - 2*X@Y.T as one matmul by augmenting the
contraction (a = -2*X):
    out[m,n] = sum_d a[m,d]*y[n,d] + 1*y2[n] + x2[m]*1 = C[m,n]

TRN2 fp32 matmul is a 2-pass HI/LO emulation (~1060ns per pass at
N=512), so instead each f32 operand is split into two bf16 parts
(a = ah + al exactly to 16 mantissa bits) and the three significant
cross products are accumulated in f32 PSUM:
    a.y ~= ah.yh + ah.yl + al.yh        (drops al*yl ~ 2^-16 |a||y|)
and the PE streams one column per cycle regardless of contraction
depth, so the two cross terms are STACKED into one K=128 matmul:
    pass 1 (K=70):  lhsT=[ah^T; ones; x2 splits] rhs=[yh^T; y2 splits; ones]
    pass 2 (K=128): lhsT=[ah^T; al^T]            rhs=[yl^T; yh^T]
Two bf16 passes are ~4x cheaper than one fp32 matmul (which is a
2-pass HI/LO emulation at 4 cycles/column) and accurate to ~1e-5
relative here (validated: max elementwise rel err 7e-6 on this input).
The y2/x2 augment lanes use exact 3-way bf16 splits multiplied by
exact ones, so the augment contributes error-free.

All operands ship pre-split/pre-transposed from the host (O(N*D)
layout prep) via SWDGE DMA pieces issued by the otherwise-idle GpSimd
engine, x-parts first so the PE starts after one piece per block.  The
kernel is raw bass: the TRN2 LDWEIGHTS encoding has only one
sync-wait slot, so Tile's auto-semaphores (which attach {PE, DVE} wait
pairs to matmuls on PSUM slot reuse) fail walrus codegen; standalone
wait_ge instructions avoid the limit.

Pipeline per core: 8 row strips of 128; per strip 4 "quads" of
[128, 2048] PSUM (4 banks); per quad 4 x 2 bf16 matmuls; quads are
evacuated PSUM->SBUF alternately by the vector and scalar engines into
4 rotating [128, 8192] strips, each written to HBM as four contiguous
1 MB quad DMAs (each gated on its single producing evacuation).  Per-core span ~110-125 us, close to the HBM
floor (36.7 MB of traffic at ~400 GB/s plus ~8 us NEFF preamble).
"""

import numpy as np
import ml_dtypes

import concourse.bass as bass
import concourse.mybir as mybir
from concourse.bass_utils import run_bass_kernel_spmd

N = 8192
D = 64
NCORES = 8
NL = N // NCORES          # 1024 rows of C per core
KB = D + 6                # 70: 64 data rows + 3 y2-split + 3 x2-split lanes (pass 1)
EPSILON = 0.1
F32 = mybir.dt.float32
BF16 = mybir.dt.bfloat16
NPBF16 = ml_dtypes.bfloat16

RSTRIPS = NL // 128       # 8 row strips of 128
QW = 2048                 # quad width: 4 PSUM banks, evacuated in one op
NQ = N // QW              # 4 quads per strip
MMW = 512                 # matmul moving width (one f32 PSUM bank)

# blockA [128, NL+N]: [ah_aug | yh_aug] rows 0-69 (70-127 zero padding
#                     so the DMA spreads over all 128 partitions)
# blockB [128, NL+N]: [ah;al  | yl;yh ]  (pass-2 stacked cross operands)
# x-part first: input piece 0 = x-part + first y-quarter in one DMA.
BLK_COLS = NL + N

_cached_nc = None


def _build_nc():
    nc = bass.Bass()

    blk_a = nc.declare_dram_parameter("blk_a", [128, BLK_COLS], BF16,
                                      isOutput=False)
    blk_b = nc.declare_dram_parameter("blk_b", [128, BLK_COLS], BF16,
                                      isOutput=False)
    c_out = nc.declare_dram_parameter("c_out", [NL, N], F32, isOutput=True)

    from contextlib import ExitStack
    ctx = ExitStack()
    with ctx:
        sb_a = ctx.enter_context(nc.sbuf_tensor("sb_a", [128, BLK_COLS], BF16))
        sb_b = ctx.enter_context(nc.sbuf_tensor("sb_b", [128, BLK_COLS], BF16))
        strips_h = [
            ctx.enter_context(nc.sbuf_tensor(f"strip{i}", [128, N], F32))
            for i in range(4)
        ]
        ps0 = ctx.enter_context(nc.psum_tensor("ps0", [128, QW], F32))
        ps1 = ctx.enter_context(nc.psum_tensor("ps1", [128, QW], F32))
        # NOTE on DMA semaphores: increments from concurrent DMAs on one
        # ring interleave (each contributes 16 in engine-completion order),
        # so a wait value is only sound if it equals the TOTAL of all
        # DMA increments that can possibly have been issued at wait time.
        # Hence: one semaphore per input piece, and one output semaphore
        # per strip buffer (consecutive users of a buffer are 4 strips
        # apart, so the wait total covers exactly the prior users).
        s_aq = [ctx.enter_context(nc.semaphore(f"s_a{j}")) for j in range(NQ)]
        s_bq = [ctx.enter_context(nc.semaphore(f"s_b{j}")) for j in range(NQ)]
        s_mm = ctx.enter_context(nc.semaphore("s_mm"))
        s_ev_d = ctx.enter_context(nc.semaphore("s_ev_d"))  # DVE evacs
        s_ev_a = ctx.enter_context(nc.semaphore("s_ev_a"))  # ACT evacs
        s_out = [ctx.enter_context(nc.semaphore(f"s_out{b}")) for b in range(4)]
        block = ctx.enter_context(nc.Block())

        strips = strips_h
        psb = [ps0, ps1]
        HW = N // 2               # half-strip output DMA width

        def piece(j):
            """Input piece j: x-part + y-quarter 0 for j=0, else quarter j."""
            lo = 0 if j == 0 else NL + j * QW
            hi = NL + (j + 1) * QW
            return lo, hi

        @block.sync
        def _(sync):
            for r in range(RSTRIPS):
                for q in range(NQ):
                    # quad q was evacuated by DVE (even q) / ACT (odd q);
                    # that engine's evac count for this quad is 2r + q//2 + 1
                    if q % 2 == 0:
                        sync.wait_ge(s_ev_d, 2 * r + q // 2 + 1)
                    else:
                        sync.wait_ge(s_ev_a, 2 * r + q // 2 + 1)
                    sync.dma_start(
                        c_out[r * 128:(r + 1) * 128, q * QW:(q + 1) * QW],
                        strips[r % 4][:, q * QW:(q + 1) * QW],
                    ).then_inc(s_out[r % 4], 16)
            for b in range(4):
                nstrips_b = len([r for r in range(RSTRIPS) if r % 4 == b])
                sync.wait_ge(s_out[b], 64 * nstrips_b)

        @block.gpsimd
        def _(gpsimd):
            # input pieces via SWDGE on the otherwise-idle GpSimd engine --
            # it starts issuing while the other engines are still in their
            # NEFF preamble.  Piece 0 carries the x-parts so the PE can
            # start after a single piece per block.
            for j in range(NQ):
                lo, hi = piece(j)
                gpsimd.dma_start(sb_a[:, lo:hi],
                                 blk_a[:, lo:hi]).then_inc(s_aq[j], 16)
                gpsimd.dma_start(sb_b[:, lo:hi],
                                 blk_b[:, lo:hi]).then_inc(s_bq[j], 16)

        @block.tensor
        def _(tensor):
            for r in range(RSTRIPS):
                ah_aug = sb_a[0:KB, r * 128:(r + 1) * 128]
                axl = sb_b[:, r * 128:(r + 1) * 128]
                for q in range(NQ):
                    k = r * NQ + q
                    if r == 0:
                        tensor.wait_ge(s_aq[q], 16)
                        tensor.wait_ge(s_bq[q], 16)
                    if k >= 2:
                        # ps[k%2] must be evacuated (quad k-2, same engine)
                        tensor.wait_ge([s_ev_d, s_ev_a][k % 2], k // 2)
                    ps = psb[k % 2]
                    for s in range(QW // MMW):
                        c0 = NL + q * QW + s * MMW
                        c1 = c0 + MMW
                        out = ps[:, s * MMW:(s + 1) * MMW]
                        tensor.matmul(out, ah_aug, sb_a[0:KB, c0:c1],
                                      start=True, stop=False)
                        mm = tensor.matmul(out, axl, sb_b[:, c0:c1],
                                           start=False, stop=True)
                    mm.then_inc(s_mm, 1)  # matmuls retire in order

        @block.vector
        def _(vector):
            for r in range(RSTRIPS):
                for q in (0, 2):
                    k = r * NQ + q
                    if r >= 4 and q == 0:
                        # strip buffer r%4: previous users are strips
                        # r-4, r-8, ... = r//4 strips, 32 incs each
                        vector.wait_ge(s_out[r % 4], 64 * (r // 4))
                    vector.wait_ge(s_mm, k + 1)
                    vector.tensor_copy(
                        strips[r % 4][:, q * QW:(q + 1) * QW], ps0[:, :]
                    ).then_inc(s_ev_d, 1)

        @block.scalar
        def _(scalar):
            for r in range(RSTRIPS):
                for q in (1, 3):
                    k = r * NQ + q
                    if r >= 4 and q == 1:
                        scalar.wait_ge(s_out[r % 4], 64 * (r // 4))
                    scalar.wait_ge(s_mm, k + 1)
                    scalar.copy(
                        strips[r % 4][:, q * QW:(q + 1) * QW], ps1[:, :]
                    ).then_inc(s_ev_a, 1)

    return nc


def _get_nc():
    global _cached_nc
    if _cached_nc is None:
        _cached_nc = _build_nc()
    return _cached_nc


def _split2(v):
    """Exact 2-way bf16 split: v ~= h + l to 16 mantissa bits."""
    h = v.astype(NPBF16).astype(np.float32)
    l = (v - h).astype(NPBF16).astype(np.float32)
    return h, l


def _split3(v):
    """3-way bf16 split: v == h + m + l to 24 mantissa bits (f32-exact)."""
    h = v.astype(NPBF16).astype(np.float32)
    r = (v - h).astype(np.float32)
    m = r.astype(NPBF16).astype(np.float32)
    l = (r - m).astype(NPBF16).astype(np.float32)
    return h, m, l


def _make_blocks(x_local, y, x2_local, y2):
    """Build blockA and blockB [128, NL+N] bf16 operands (x-part first)."""
    a = (-2.0 * x_local).astype(np.float32)
    ah, al = _split2(a)
    yhi, ylo = _split2(y)
    y2h, y2m, y2l = _split3(y2)
    x2h, x2m, x2l = _split3(x2_local)

    blk_a = np.zeros((128, BLK_COLS), dtype=np.float32)
    # ah_aug (x-part): data, ones (match y2 lanes), x2 splits
    blk_a[:D, :NL] = ah.T
    blk_a[D + 0:D + 3, :NL] = 1.0
    blk_a[D + 3, :NL] = x2h
    blk_a[D + 4, :NL] = x2m
    blk_a[D + 5, :NL] = x2l
    # yh_aug (y-part): data, y2 splits, ones
    blk_a[:D, NL:] = yhi.T
    blk_a[D + 0, NL:] = y2h
    blk_a[D + 1, NL:] = y2m
    blk_a[D + 2, NL:] = y2l
    blk_a[D + 3:D + 6, NL:] = 1.0

    blk_b = np.empty((128, BLK_COLS), dtype=np.float32)
    # stacked cross operands: sum_k lhsT[k]*rhs[k] = ah.yl + al.yh
    blk_b[:D, :NL] = ah.T
    blk_b[D:, :NL] = al.T
    blk_b[:D, NL:] = ylo.T
    blk_b[D:, NL:] = yhi.T
    return blk_a.astype(NPBF16), blk_b.astype(NPBF16)


def _lse_cols(a):
    """jax.nn.logsumexp(a, axis=0) in f32 numpy, with jax's non-finite
    amax guard."""
    amax = np.max(a, axis=0)
    amax_f = np.where(np.isfinite(amax), amax, np.float32(0)).astype(np.float32)
    with np.errstate(over="ignore", invalid="ignore", divide="ignore"):
        s = np.sum(np.exp((a - amax_f[None, :]).astype(np.float32)),
                   axis=0).astype(np.float32)
        return (np.log(s) + amax_f).astype(np.float32)


def _sinkhorn_host(C, max_iter=100):
    """Faithful numpy replication of the reference Sinkhorn loop, used only
    if the saturation fast path does not apply."""
    n = C.shape[0]
    log_K = (-C / np.float32(EPSILON)).astype(np.float32)
    log_a = np.float32(np.log(np.float32(1.0 / n) + np.float32(1e-8)))
    log_b = log_a
    u = np.zeros(n, np.float32)
    v = np.zeros(n, np.float32)
    with np.errstate(over="ignore", invalid="ignore", divide="ignore"):
        for _ in range(max_iter):
            u_new = np.exp((log_a - _lse_cols(log_K + v[:, None])).astype(np.float32))
            v_new = np.exp((log_b - _lse_cols(log_K.T + u_new[:, None])).astype(np.float32))
            if (np.array_equal(u_new, u, equal_nan=True)
                    and np.array_equal(v_new, v, equal_nan=True)):
                u, v = u_new, v_new
                break  # bitwise fixed point: remaining iterations are no-ops
            u, v = u_new, v_new
        plan = np.exp((log_K + v[:, None] + u[None, :]).astype(np.float32))
    return plan


def kernel(**inputs):
    x = np.ascontiguousarray(np.asarray(inputs["source"]), dtype=np.float32)
    y = np.ascontiguousarray(np.asarray(inputs["target"]), dtype=np.float32)
    assert x.shape == (N, D) and y.shape == (N, D)

    x2 = np.sum(x * x, axis=-1, dtype=np.float32)
    y2 = np.sum(y * y, axis=-1, dtype=np.float32)

    nc = _get_nc()
    in_maps = []
    for d in range(NCORES):
        blk_a, blk_b = _make_blocks(x[d * NL:(d + 1) * NL], y,
                                    x2[d * NL:(d + 1) * NL], y2)
        in_maps.append({"blk_a": blk_a, "blk_b": blk_b})
    res = run_bass_kernel_spmd(nc, in_maps, core_ids=list(range(NCORES)))
    C = np.concatenate([r["c_out"] for r in res.results], axis=0)

    # Saturation fast path, rigorous bound: for every column j,
    #   log_u[j] = log_a - LSE_i(-10 C[i,j]) >= log_a + 10*min(C) - ln(N).
    # If that exceeds 100 (f32 exp overflows to +inf at 88.73), then
    # u == +inf everywhere, so v == 0 (LSE of a +inf column is +inf
    # under jax's guard) and (u, v) is a bitwise fixed point
    # => plan = exp(finite + 0 + inf) = +inf everywhere.
    log_a = float(np.log(np.float32(1.0 / N) + np.float32(1e-8)))
    bound = log_a + (1.0 / EPSILON) * float(C.min()) - float(np.log(N))
    if bound > 100.0:
        plan = np.full((N, N), np.inf, dtype=np.float32)
    else:
        plan = _sinkhorn_host(C)
    return plan, C
